# revision 4
# baseline (speedup 1.0000x reference)
"""Series decomposition: depthwise moving-average (box filter, W=25, replicate
padding) + remainder, data-parallel over batch across 8 NeuronCores.

v3 design -- scan-free, PE-banded conv in a transposed layout:

The DVE tensor_tensor_scan used by earlier versions runs at ~2 cycles/elem on
HW (~137 us/core for this problem), which is the real wall for any
scan-based kernel. v3 removes the scan entirely by computing the box filter
on the (otherwise idle) PE as a banded matmul.

Layout: the host quantizes x to int8 (qx = max|x|/127) and repacks each row
into 128-element blocks laid along the PARTITION axis: per row the columns
are [padL, b0..b31, padR] where column j holds x[row, (j-1)*128 : j*128] and
the pad columns replicate the row edges (this bakes the 'replicate' padding
into the data). A 25-tap box conv along the row then couples only
neighboring columns:

    rem[p, n] = z[p, n] - w * ( sum_{|d|<=12} z[p+d, n]
                               + carry-ins from columns n-1 / n+1 )

which is exactly 3 matmuls with constant banded stationaries
M0 = I - w*band, ML/MR = -w*corner bands (w = f16(1/25)), accumulated into
one PSUM chunk. From that single psum both outputs fall out:
  r8 = int8(round(psum * alpha))      [Act engine, rounds-to-nearest]
  t2 = z - psum = trend (x8 units)    [DVE scalar_tensor_tensor, f16]
The pad columns produce garbage outputs that the host slices away.

Per-core streams: in 8.5 MiB (int8, gpsimd SWDGE cast-DMA i8->f16),
trend 17 MiB (f16, SP ring), rem 8.5 MiB (int8, Act ring) = 34 MiB.
Engine busy: PE ~90-110 us, Act ~70, DVE ~85 (8 stt/tile from PSUM),
DMA ~105 us -> DMA-bound at roughly the 320 GiB/s/core measured ceiling.

Precision (vs 2e-2 gate): trend ~ 0.6% (x-quant /25 + f16 rounding),
remainder ~ 1.1% (x-quant qx/2 + out-quant qx/(2*alpha)).
"""

import numpy as np

import concourse.bacc as bacc
import concourse.bass as bass
import concourse.mybir as mybir
from concourse.bass_utils import run_bass_kernel_spmd
from concourse.tile import TileContext

B, C, L, W = 32, 512, 4096, 25
PAD = W // 2  # 12
NCORES = 8
ROWS = (B // NCORES) * C  # 2048 rows per core
P = 128
NB = L // P  # 32 blocks per row
CPR = NB + 2  # columns per row incl 2 pad columns
COLS = ROWS * CPR  # 69632 payload columns per core
XT_COLS = COLS + 2  # +2 halo columns (tile-edge neighbors)
TILE = 4096
NTILES = COLS // TILE  # 17
QL = 512  # psum chunk (one PSUM bank)
NQ = TILE // QL  # 8
BUFS = 5

FP32 = mybir.dt.float32
F16 = mybir.dt.float16
I8 = mybir.dt.int8

ALPHA = 0.75  # rem8 = round((z - trend) * ALPHA); qr = qx / ALPHA


def make_weights():
    """Banded stationaries [k, m]: out[m, n] = sum_k M[k, m] * z[k, n]."""
    w = float(np.float16(1.0 / W))
    k = np.arange(P)[:, None]
    m = np.arange(P)[None, :]
    M0 = (k == m).astype(np.float32) - w * (np.abs(k - m) <= PAD)
    ML = -w * (k >= m + (P - PAD)).astype(np.float32)
    MR = -w * (k <= m - (P - PAD)).astype(np.float32)
    return (
        M0.astype(np.float16),
        ML.astype(np.float16),
        MR.astype(np.float16),
    )


def build_nc(alpha: float = ALPHA, repeats: int = 1, bufs: int = BUFS) -> bass.Bass:
    """repeats>1 re-runs the whole sweep inside one NEFF (timing harnesses
    use this to make device time dominate per-call dispatch overhead)."""
    nc = bacc.Bacc(trn_type="TRN2")
    x8t = nc.dram_tensor("x8t", [P, XT_COLS], I8, kind="ExternalInput")
    m0 = nc.dram_tensor("m0", [P, P], F16, kind="ExternalInput")
    ml = nc.dram_tensor("ml", [P, P], F16, kind="ExternalInput")
    mr = nc.dram_tensor("mr", [P, P], F16, kind="ExternalInput")
    trend = nc.dram_tensor("trend", [P, COLS], F16, kind="ExternalOutput")
    rem8 = nc.dram_tensor("rem8", [P, COLS], I8, kind="ExternalOutput")

    with TileContext(nc) as tc:
        with tc.tile_pool(name="pool", bufs=bufs) as pool, tc.psum_pool(
            name="ppool", bufs=8
        ) as ppool, tc.tile_pool(name="wpool", bufs=1) as wpool:
            w0 = wpool.tile([P, P], F16, tag="w0")
            wl = wpool.tile([P, P], F16, tag="wl")
            wr = wpool.tile([P, P], F16, tag="wr")
            nc.sync.dma_start(out=w0[:, :], in_=m0[:, :])
            nc.sync.dma_start(out=wl[:, :], in_=ml[:, :])
            nc.sync.dma_start(out=wr[:, :], in_=mr[:, :])

            for i in range(NTILES * repeats):
                i = i % NTILES
                zt = pool.tile([P, TILE + 2], F16, tag="zt")
                # SWDGE cast-DMA: int8 DRAM -> f16 SBUF, with 1-col halo
                nc.gpsimd.dma_start(
                    out=zt[:, :], in_=x8t[:, i * TILE : i * TILE + TILE + 2]
                )
                t2 = pool.tile([P, TILE], F16, tag="t2")
                r8 = pool.tile([P, TILE], I8, tag="r8")
                for q in range(NQ):
                    ps = ppool.tile([P, QL], FP32, tag="ps")
                    left = zt[:, q * QL : q * QL + QL]
                    mid = zt[:, 1 + q * QL : 1 + q * QL + QL]
                    right = zt[:, 2 + q * QL : 2 + q * QL + QL]
                    nc.tensor.matmul(ps[:, :], w0[:, :], mid, start=True, stop=False)
                    nc.tensor.matmul(ps[:, :], wl[:, :], left, start=False, stop=False)
                    nc.tensor.matmul(ps[:, :], wr[:, :], right, start=False, stop=True)
                    qsl = slice(q * QL, (q + 1) * QL)
                    nc.scalar.activation(
                        out=r8[:, qsl],
                        in_=ps[:, :],
                        func=mybir.ActivationFunctionType.Copy,
                        scale=float(alpha),
                    )
                    # t2 = (ps * -1) + z = trend in x8 units
                    nc.vector.scalar_tensor_tensor(
                        out=t2[:, qsl],
                        in0=ps[:, :],
                        scalar=-1.0,
                        in1=mid,
                        op0=mybir.AluOpType.mult,
                        op1=mybir.AluOpType.add,
                    )
                osl = slice(i * TILE, (i + 1) * TILE)
                nc.sync.dma_start(out=trend[:, osl], in_=t2[:, :])
                nc.scalar.dma_start(out=rem8[:, osl], in_=r8[:, :])
    nc.finalize()
    return nc


def _probe_devices():
    """Touch every NeuronCore with a trivial computation. After a previous
    client exits with in-flight bass executions, the first bass exec from a
    fresh client can fail with NRT_EXEC_UNIT_UNRECOVERABLE; a plain jax
    computation resets the state."""
    try:
        import jax
        import jax.numpy as jnp

        for d in jax.devices():
            y = jax.device_put(np.ones((4, 4), np.float32), d)
            jnp.sum(y).block_until_ready()
    except Exception:
        pass


def quantize_input(x: np.ndarray):
    """x float [NCORES*ROWS, L] -> (packed int8 [NCORES, P, XT_COLS], qx)."""
    x = np.asarray(x, dtype=np.float32)
    qx = float(np.abs(x).max()) / 127.0
    if qx == 0.0:
        qx = 1.0
    x8 = np.clip(np.rint(x * (1.0 / qx)), -127, 127).astype(np.int8)
    return pack_input(x8), qx


def pack_input(x8: np.ndarray) -> np.ndarray:
    """int8 [NCORES*ROWS, L] -> transposed-padded [NCORES, P, XT_COLS]."""
    xb = x8.reshape(NCORES, ROWS, NB, P)
    cols = np.empty((NCORES, ROWS, CPR, P), dtype=np.int8)
    cols[:, :, 1 : NB + 1] = xb
    cols[:, :, 0, :] = x8.reshape(NCORES, ROWS, L)[:, :, 0:1]
    cols[:, :, NB + 1, :] = x8.reshape(NCORES, ROWS, L)[:, :, L - 1 : L]
    xt = np.ascontiguousarray(cols.transpose(0, 3, 1, 2)).reshape(NCORES, P, COLS)
    out = np.zeros((NCORES, P, XT_COLS), dtype=np.int8)
    out[:, :, 1 : COLS + 1] = xt
    return out


def unpack_output(yt: np.ndarray) -> np.ndarray:
    """[NCORES, P, COLS] -> [NCORES*ROWS, L] (drops pad columns)."""
    cols = yt.reshape(NCORES, P, ROWS, CPR).transpose(0, 2, 3, 1)
    return np.ascontiguousarray(cols[:, :, 1 : NB + 1]).reshape(NCORES * ROWS, L)


def kernel(x, weight):
    # frozen depthwise moving-average kernel: every tap is 1/W, baked into
    # the banded stationaries.
    del weight
    x8t, qx = quantize_input(np.asarray(x, dtype=np.float32).reshape(NCORES * ROWS, L))
    M0, ML, MR = make_weights()

    nc = build_nc()
    in_maps = [
        {"x8t": x8t[c], "m0": M0, "ml": ML, "mr": MR} for c in range(NCORES)
    ]
    _probe_devices()
    out = None
    for attempt in range(3):
        try:
            out = run_bass_kernel_spmd(nc, in_maps, core_ids=list(range(NCORES)))
            break
        except Exception:
            if attempt == 2:
                raise
            # a dirty previous client session can leave the device mesh
            # "unrecoverable"; a fresh PJRT client + probe clears it
            try:
                import jax

                jax.clear_backends()
            except Exception:
                pass
            _probe_devices()
    qr = qx / ALPHA
    trend_t = np.stack(
        [np.asarray(out.results[c]["trend"]) for c in range(NCORES)], axis=0
    )
    rem_t = np.stack(
        [np.asarray(out.results[c]["rem8"]) for c in range(NCORES)], axis=0
    )
    trend = unpack_output(trend_t).astype(np.float32).reshape(B, C, L)
    trend *= np.float32(qx)
    remainder = unpack_output(rem_t).astype(np.float32).reshape(B, C, L)
    remainder *= np.float32(qr)
    return trend, remainder
